# revision 2
# baseline (speedup 1.0000x reference)
"""Depth-modulated 3x3 conv (DepthConv) for Trainium2, 8-way batch-parallel.

out(b,o,h,w) = sum_{c,i,j} W[o,c,i,j] * x[b,c,h+i-1,w+j-1]
               * exp(-8.3*|d[b,h,w] - d[b,h+i-1,w+j-1]|)

Strategy (per core = one batch item):
  - Pixels are matmul OUTPUT partitions: 32 tiles of 128 px (2 rows).
  - For each row-shift i in {0,1,2}: stationary lhsT = x[cin_chunk, 128 px
    shifted by (i-1)*64] (bf16), moving rhs = W[cin_chunk, (j,o)=192] (bf16),
    4 cin chunks accumulate in PSUM -> y3[p, i, (j,o)].
  - Gates exp(-8.3|dd|) are precomputed on the HOST (g[p, t, 9] f32, zero at
    invalid taps, one at center) and consumed with a step-0 broadcast AP
    along cout: one DVE tensor_tensor per tile gates the whole casted tile.
  - The i-sum runs on the otherwise-idle Pool engine (nc.gpsimd) as two
    tensor_tensor adds, keeping DVE under the PE period.
  - The leftover w-shift (j-1 = +-1) and the sum over j happen on the host:
    out[q] = P0[q-1] + P1[q] + P2[q+1] over the three j-aligned streams.
  - Input DMAs are issued on the sync queue in consumption order (w, then
    x column-blocks k-major, gates after the first block) so the PE starts
    ~9us in instead of waiting for the full 4.3MB of x.
  - ~100 dummy matmuls on a memset scratch tile run during the DMA lead-in
    to hold the PE clock gate (HAM) at 2.4GHz before real work arrives.
"""
import os
import sys
sys.path.insert(0, '/opt/trn_rl_repo')

import numpy as np
import ml_dtypes

import concourse.bass as bass
import concourse.tile as tile
from concourse import bacc, mybir
from concourse.bass_utils import run_bass_kernel_spmd

F32 = mybir.dt.float32
BF16 = mybir.dt.bfloat16

B, CIN, H, W = 8, 512, 64, 64
COUT, K = 64, 3
ALPHA = 8.3
NPX = H * W            # 4096
NT = NPX // 128        # 32 pixel tiles
KC = CIN // 128        # 4 cin chunks
XCOLS = NPX + 128      # 64 guard + 4096 + 64 guard
HG = 4                 # tiles per half-group (output DMA granularity)
NH = NT // HG          # 8 half-groups
# consumption-ordered x column blocks (each DMA'd per cin chunk)
XBLOCKS = [(0, 384), (384, 1152), (1152, 2176), (2176, 3200), (3200, 4224)]

_cache = {}


def build_nc():
    nc = bacc.Bacc("TRN2", target_bir_lowering=False, debug=False, num_devices=B)
    x_d = nc.dram_tensor("x", [128, KC, XCOLS], BF16, kind="ExternalInput").ap()
    w_d = nc.dram_tensor("w", [128, KC, 3, 192], BF16, kind="ExternalInput").ap()
    g_d = nc.dram_tensor("g", [128, NT, 9], F32, kind="ExternalInput").ap()
    # three j-aligned partial streams; host applies the +-1 pixel shift + sum
    out_d = nc.dram_tensor("out", [NPX, 3, COUT], F32, kind="ExternalOutput").ap()

    with tile.TileContext(nc) as tc:
        with tc.tile_pool(name="const", bufs=1) as cpool, \
             tc.tile_pool(name="work", bufs=3) as wpool, \
             tc.tile_pool(name="phg", bufs=4) as hgpool, \
             tc.tile_pool(name="psum", bufs=3, space="PSUM") as ppool, \
             tc.tile_pool(name="pdum", bufs=1, space="PSUM") as dpool:

            # PE pre-warm: dummy matmuls on a memset scratch tile keep the
            # HAM clock gate busy while the real inputs stream in.
            scratch = cpool.tile([128, 128], BF16)
            nc.gpsimd.memset(scratch[:], 0.0)
            ps_dummy = dpool.tile([128, 512], F32)
            for _ in range(100):
                nc.tensor.matmul(ps_dummy[:, 0:128], scratch[:], scratch[:],
                                 start=True, stop=True)

            # input DMAs, consumption order, all on the sync queue
            w_sb = cpool.tile([128, KC, 3, 192], BF16)
            x_sb = cpool.tile([128, KC, XCOLS], BF16)
            g_sb = cpool.tile([128, NT, 9], F32)
            nc.sync.dma_start(w_sb[:], w_d[:])
            for k in range(KC):
                c0, c1 = XBLOCKS[0]
                nc.sync.dma_start(x_sb[:, k, c0:c1], x_d[:, k, c0:c1])
            nc.sync.dma_start(g_sb[:], g_d[:])
            for c0, c1 in XBLOCKS[1:]:
                for k in range(KC):
                    nc.sync.dma_start(x_sb[:, k, c0:c1], x_d[:, k, c0:c1])

            for hg in range(NH):
                p_hg = hgpool.tile([128, HG, 3, COUT], F32, tag="phg")
                for th in range(HG):
                    t = hg * HG + th
                    ps = ppool.tile([128, 4, 256], F32, tag="ps")
                    for i in range(3):
                        base = 64 + t * 128 + (i - 1) * 64
                        for k in range(KC):
                            nc.tensor.matmul(
                                ps[:, i, 0:192],
                                x_sb[:, k, base:base + 128],
                                w_sb[:, k, i, :],
                                start=(k == 0), stop=(k == KC - 1),
                            )
                    # ACT casts PSUM -> bf16 SBUF
                    y_bf = wpool.tile([128, 3, 192], BF16, tag="ybf")
                    nc.scalar.copy(y_bf[:], ps[:, 0:3, 0:192])
                    # DVE gates the tile; gate broadcast along cout (step-0)
                    tmp = wpool.tile([128, 3, 3, COUT], BF16, tag="tmp")
                    g_t = (g_sb[:, t, :, None]
                           .broadcast_to([128, 9, COUT])
                           .rearrange("p (i j) o -> p i j o", i=3))
                    nc.vector.tensor_tensor(
                        tmp[:], y_bf[:].rearrange("p i (j o) -> p i j o", j=3),
                        g_t, op=mybir.AluOpType.mult)
                    # i-sum on the Pool engine
                    s1 = wpool.tile([128, 3, COUT], BF16, tag="s1")
                    nc.gpsimd.tensor_tensor(s1[:], tmp[:, 0], tmp[:, 1],
                                            op=mybir.AluOpType.add)
                    nc.gpsimd.tensor_tensor(p_hg[:, th, :, :], s1[:], tmp[:, 2],
                                            op=mybir.AluOpType.add)

                # one contiguous DMA per half-group; host sums the 3 streams
                dst = out_d[512 * hg:512 * (hg + 1)].rearrange(
                    "(t p) j o -> p t j o", p=128)
                nc.sync.dma_start(dst, p_hg[:])

    nc.compile()
    return nc


def prep_inputs(input, depth, weight):
    """Host-side relayout: returns per-core in_maps."""
    # x: (B, 512, 64, 64) -> [128, KC, XCOLS] bf16 with zero guards
    xr = input.reshape(B, KC, 128, NPX).transpose(0, 2, 1, 3)  # [B,128,KC,NPX]
    x_all = np.zeros((B, 128, KC, XCOLS), dtype=ml_dtypes.bfloat16)
    x_all[:, :, :, 64:64 + NPX] = xr.astype(ml_dtypes.bfloat16)

    # w: (64, 512, 3, 3) -> [128, KC, 3(i), 192(j*64+o)] bf16
    wr = weight.reshape(COUT, KC, 128, 3, 3)
    w_dev = wr.transpose(2, 1, 3, 4, 0).reshape(128, KC, 3, 192)
    w_dev = np.ascontiguousarray(w_dev).astype(ml_dtypes.bfloat16)

    # gates, consumed at y-alignment q' (pre-shifted by 1-j):
    #   g_ij[q'] = exp(-a*|d[q] - d[q + off_ij]|), q = q' + 1 - j,
    #   off_ij = 64*(i-1) + (j-1); invalid taps -> exactly 0
    d = depth.reshape(B, H, W).astype(np.float32)
    dflat = d.reshape(B, NPX)
    g_all = np.zeros((B, 128, NT, 9), dtype=np.float32)
    qp = np.arange(NPX)
    for i in range(3):
        for j in range(3):
            q = qp + 1 - j
            q_ok = (q >= 0) & (q < NPX)
            qc = np.clip(q, 0, NPX - 1)
            h_q, w_q = qc // W, qc % W
            hn, wn = h_q + i - 1, w_q + j - 1
            n_ok = q_ok & (hn >= 0) & (hn < H) & (wn >= 0) & (wn < W)
            hnc = np.clip(hn, 0, H - 1)
            wnc = np.clip(wn, 0, W - 1)
            a = dflat[:, qc]                      # d at out pixel
            bV = d[:, hnc, wnc]                   # d at neighbor
            gv = np.exp(-ALPHA * np.abs(a - bV)) * n_ok[None, :]
            # [B, NPX] -> [B, p=(q'%128), t=(q'//128)] ; q' = h*64+w
            g_all[:, :, :, 3 * i + j] = (
                gv.reshape(B, 32, 128).transpose(0, 2, 1))

    return [
        {"x": x_all[b], "w": w_dev, "g": g_all[b]}
        for b in range(B)
    ]


def kernel(input, depth, weight):
    input = np.asarray(input, dtype=np.float32)
    depth = np.asarray(depth, dtype=np.float32)
    weight = np.asarray(weight, dtype=np.float32)

    if "nc" not in _cache:
        _cache["nc"] = build_nc()
    nc = _cache["nc"]

    in_maps = prep_inputs(input, depth, weight)
    kwargs = {}
    if os.environ.get("KERNEL_TRACE") == "1":
        kwargs = dict(trace=True, trace_cores=list(range(B)))
    res = run_bass_kernel_spmd(nc, in_maps, core_ids=list(range(B)), **kwargs)
    _cache["last_results"] = res
    # combine the three j-aligned streams: out[q] = P0[q-1] + P1[q] + P2[q+1]
    outs = []
    for b in range(B):
        p3 = res.results[b]["out"]          # [NPX, 3, COUT]
        o = p3[:, 1, :].astype(np.float32).copy()
        o[1:] += p3[:-1, 0, :]
        o[:-1] += p3[1:, 2, :]
        outs.append(o.T.reshape(COUT, H, W))
    return np.stack(outs).astype(np.float32)


if __name__ == "__main__":
    rng = np.random.default_rng(0)
    x = rng.standard_normal((B, CIN, H, W), dtype=np.float32)
    d = rng.random((B, 1, H, W), dtype=np.float32)
    w = (rng.random((COUT, CIN, 3, 3), dtype=np.float32) - 0.5) * 0.08
    o = kernel(x, d, w)
    print(o.shape, o.dtype)


# revision 4
# speedup vs baseline: 1.1095x; 1.1095x over previous
"""Depth-modulated 3x3 conv (DepthConv) for Trainium2, 8-way batch-parallel.

out(b,o,h,w) = sum_{c,i,j} W[o,c,i,j] * x[b,c,h+i-1,w+j-1]
               * exp(-8.3*|d[b,h,w] - d[b,h+i-1,w+j-1]|)

Strategy (per core = one batch item):
  - Pixels are matmul OUTPUT partitions: 32 tiles of 128 px (2 rows).
  - For each row-shift i in {0,1,2}: stationary lhsT = x[cin_chunk, 128 px
    shifted by (i-1)*64] (bf16), moving rhs = W[cin_chunk, (j,o)=192] (bf16),
    4 cin chunks accumulate in PSUM -> y3[p, i, (j,o)].
  - Gates exp(-8.3|dd|) are precomputed AND replicated along cout on the
    HOST (g_rep[p, t, 9, 64] bf16, zero at invalid taps, one at center), so
    the per-tile DVE multiply runs in 2x bf16 mode with plain APs and no
    on-device replication build is needed.
  - i-sum: first add on the otherwise-idle Pool engine (f32 output = its
    fast path), second add on DVE but EMITTED one tile later so the DVE
    FIFO never waits on Pool (period stays PE-limited).
  - The leftover w-shift (j-1 = +-1) and the sum over j happen on the host:
    out[q] = P0[q-1] + P1[q] + P2[q+1] over the three j-aligned streams.
  - All input DMAs are issued on the sync queue in consumption order
    (w, x column-blocks k-major, gate pair-chunks interleaved) so the PE
    starts ~10us in instead of waiting for the full 9.6MB of input.
  - ~60 dummy matmuls on a memset scratch tile run during the DMA lead-in
    to trip the PE clock gate (HAM) to 2.4GHz before real work arrives.
"""
import os
import sys
sys.path.insert(0, '/opt/trn_rl_repo')

import numpy as np
import ml_dtypes

import concourse.bass as bass
import concourse.tile as tile
from concourse import bacc, mybir
from concourse.bass_utils import run_bass_kernel_spmd

F32 = mybir.dt.float32
BF16 = mybir.dt.bfloat16

B, CIN, H, W = 8, 512, 64, 64
COUT, K = 64, 3
ALPHA = 8.3
NPX = H * W            # 4096
NT = NPX // 128        # 32 pixel tiles
KC = CIN // 128        # 4 cin chunks
XCOLS = NPX + 128      # 64 guard + 4096 + 64 guard
HG = 4                 # tiles per half-group (output DMA granularity)
NH = NT // HG          # 8 half-groups
# consumption-ordered x column blocks (each DMA'd per cin chunk)
XBLOCKS = [(0, 384), (384, 1152), (1152, 2176), (2176, 3200), (3200, 4224)]

_cache = {}


def build_nc():
    nc = bacc.Bacc("TRN2", target_bir_lowering=False, debug=False, num_devices=B)
    x_d = nc.dram_tensor("x", [128, KC, XCOLS], BF16, kind="ExternalInput").ap()
    w_d = nc.dram_tensor("w", [128, KC, 3, 192], BF16, kind="ExternalInput").ap()
    g_d = nc.dram_tensor("g", [128, NT, 9, COUT], BF16, kind="ExternalInput").ap()
    # three j-aligned partial streams; host applies the +-1 pixel shift + sum
    out_d = nc.dram_tensor("out", [NPX, 3, COUT], F32, kind="ExternalOutput").ap()

    with tile.TileContext(nc) as tc:
        with tc.tile_pool(name="const", bufs=1) as cpool, \
             tc.tile_pool(name="work", bufs=3) as wpool, \
             tc.tile_pool(name="phg", bufs=5) as hgpool, \
             tc.tile_pool(name="psum", bufs=3, space="PSUM") as ppool, \
             tc.tile_pool(name="pdum", bufs=1, space="PSUM") as dpool:

            # PE pre-warm: dummy matmuls on a memset scratch tile keep the
            # HAM clock gate busy while the real inputs stream in.
            scratch = cpool.tile([128, 128], BF16)
            nc.gpsimd.memset(scratch[:], 0.0)
            ps_dummy = dpool.tile([128, 512], F32)
            for _ in range(60):
                nc.tensor.matmul(ps_dummy[:, 0:64], scratch[:], scratch[:, 0:64],
                                 start=True, stop=True)

            # input DMAs, consumption order, all on the sync queue
            w_sb = cpool.tile([128, KC, 3, 192], BF16)
            x_sb = cpool.tile([128, KC, XCOLS], BF16)
            g_sb = cpool.tile([128, NT, 9, COUT], BF16)

            def x_block(bi):
                c0, c1 = XBLOCKS[bi]
                for k in range(KC):
                    nc.sync.dma_start(x_sb[:, k, c0:c1], x_d[:, k, c0:c1])

            def g_pairs(q0, q1):
                for q in range(q0, q1):
                    nc.sync.dma_start(g_sb[:, 2 * q:2 * q + 2],
                                      g_d[:, 2 * q:2 * q + 2])

            nc.sync.dma_start(w_sb[:], w_d[:])
            x_block(0)            # x cols [0,384)     tiles 0-2
            g_pairs(0, 2)         # gates tiles 0-3
            x_block(1)            # x cols [384,1152)  tiles 3-7
            g_pairs(2, 4)         # gates tiles 4-7
            x_block(2)            # x cols [1152,2176) tiles 8-15
            g_pairs(4, 8)         # gates tiles 8-15
            x_block(3)            # x cols [2176,3200) tiles 16-23
            g_pairs(8, 12)        # gates tiles 16-23
            x_block(4)            # x cols [3200,4224) tiles 24-31
            g_pairs(12, 16)       # gates tiles 24-31

            # steady pipeline: PE -> ACT cast -> DVE gate-mult -> Pool add1
            # -> DVE add2 (deferred one tile) -> per-half-group DMA out
            pend = None           # (tile idx, s1, tmp) awaiting add2
            phg_tiles = {}

            def emit_add2(pt, ps1, ptmp):
                hg_i, th_i = pt // HG, pt % HG
                if hg_i not in phg_tiles:
                    p_hg = hgpool.tile([128, HG, 3, COUT], F32, tag="phg",
                                       name=f"p_hg{hg_i}")
                    phg_tiles[hg_i] = p_hg
                nc.vector.tensor_tensor(phg_tiles[hg_i][:, th_i, :, :],
                                        ps1[:], ptmp[:, 2],
                                        op=mybir.AluOpType.add)
                if th_i == HG - 1:
                    dst = out_d[512 * hg_i:512 * (hg_i + 1)].rearrange(
                        "(t p) j o -> p t j o", p=128)
                    nc.sync.dma_start(dst, phg_tiles[hg_i][:])

            for t in range(NT):
                ps = ppool.tile([128, 4, 256], F32, tag="ps")
                for i in range(3):
                    base = 64 + t * 128 + (i - 1) * 64
                    for k in range(KC):
                        nc.tensor.matmul(
                            ps[:, i, 0:192],
                            x_sb[:, k, base:base + 128],
                            w_sb[:, k, i, :],
                            start=(k == 0), stop=(k == KC - 1),
                        )
                # ACT casts PSUM -> bf16 SBUF; DVE per-tile mult runs 2x bf16
                y_bf = wpool.tile([128, 3, 192], BF16, tag="ybf")
                nc.scalar.copy(y_bf[:], ps[:, 0:3, 0:192])
                tmp = wpool.tile([128, 3, 3, COUT], BF16, tag="tmp")
                nc.vector.tensor_tensor(
                    tmp[:], y_bf[:].rearrange("p i (j o) -> p i j o", j=3),
                    g_sb[:, t, :, :].rearrange("p (i j) o -> p i j o", i=3),
                    op=mybir.AluOpType.mult)
                s1 = wpool.tile([128, 3, COUT], F32, tag="s1")
                nc.gpsimd.tensor_tensor(s1[:], tmp[:, 0], tmp[:, 1],
                                        op=mybir.AluOpType.add)
                if pend is not None:
                    emit_add2(*pend)
                pend = (t, s1, tmp)
            emit_add2(*pend)

    nc.compile()
    return nc


def prep_inputs(input, depth, weight):
    """Host-side relayout: returns per-core in_maps."""
    # x: (B, 512, 64, 64) -> [128, KC, XCOLS] bf16 with zero guards
    xr = input.reshape(B, KC, 128, NPX).transpose(0, 2, 1, 3)  # [B,128,KC,NPX]
    x_all = np.zeros((B, 128, KC, XCOLS), dtype=ml_dtypes.bfloat16)
    x_all[:, :, :, 64:64 + NPX] = xr.astype(ml_dtypes.bfloat16)

    # w: (64, 512, 3, 3) -> [128, KC, 3(i), 192(j*64+o)] bf16
    wr = weight.reshape(COUT, KC, 128, 3, 3)
    w_dev = wr.transpose(2, 1, 3, 4, 0).reshape(128, KC, 3, 192)
    w_dev = np.ascontiguousarray(w_dev).astype(ml_dtypes.bfloat16)

    # gates, consumed at y-alignment q' (pre-shifted by 1-j):
    #   g_ij[q'] = exp(-a*|d[q] - d[q + off_ij]|), q = q' + 1 - j,
    #   off_ij = 64*(i-1) + (j-1); invalid taps -> exactly 0
    d = depth.reshape(B, H, W).astype(np.float32)
    dflat = d.reshape(B, NPX)
    g_all = np.zeros((B, 128, NT, 9), dtype=np.float32)
    qp = np.arange(NPX)
    for i in range(3):
        for j in range(3):
            q = qp + 1 - j
            q_ok = (q >= 0) & (q < NPX)
            qc = np.clip(q, 0, NPX - 1)
            h_q, w_q = qc // W, qc % W
            hn, wn = h_q + i - 1, w_q + j - 1
            n_ok = q_ok & (hn >= 0) & (hn < H) & (wn >= 0) & (wn < W)
            hnc = np.clip(hn, 0, H - 1)
            wnc = np.clip(wn, 0, W - 1)
            a = dflat[:, qc]                      # d at out pixel
            bV = d[:, hnc, wnc]                   # d at neighbor
            gv = np.exp(-ALPHA * np.abs(a - bV)) * n_ok[None, :]
            # [B, NPX] -> [B, p=(q'%128), t=(q'//128)] ; q' = h*64+w
            g_all[:, :, :, 3 * i + j] = (
                gv.reshape(B, 32, 128).transpose(0, 2, 1))
    # replicate along cout so DVE consumes plain (non-broadcast) bf16 APs
    g_rep = np.ascontiguousarray(
        np.broadcast_to(g_all[..., None], (B, 128, NT, 9, COUT))
    ).astype(ml_dtypes.bfloat16)

    return [
        {"x": x_all[b], "w": w_dev, "g": g_rep[b]}
        for b in range(B)
    ]


def kernel(input, depth, weight):
    input = np.asarray(input, dtype=np.float32)
    depth = np.asarray(depth, dtype=np.float32)
    weight = np.asarray(weight, dtype=np.float32)

    if "nc" not in _cache:
        _cache["nc"] = build_nc()
    nc = _cache["nc"]

    in_maps = prep_inputs(input, depth, weight)
    kwargs = {}
    if os.environ.get("KERNEL_TRACE") == "1":
        kwargs = dict(trace=True, trace_cores=list(range(B)))
    res = run_bass_kernel_spmd(nc, in_maps, core_ids=list(range(B)), **kwargs)
    _cache["last_results"] = res
    # combine the three j-aligned streams: out[q] = P0[q-1] + P1[q] + P2[q+1]
    outs = []
    for b in range(B):
        p3 = res.results[b]["out"]          # [NPX, 3, COUT]
        o = p3[:, 1, :].astype(np.float32).copy()
        o[1:] += p3[:-1, 0, :]
        o[:-1] += p3[1:, 2, :]
        outs.append(o.T.reshape(COUT, H, W))
    return np.stack(outs).astype(np.float32)


if __name__ == "__main__":
    rng = np.random.default_rng(0)
    x = rng.standard_normal((B, CIN, H, W), dtype=np.float32)
    d = rng.random((B, 1, H, W), dtype=np.float32)
    w = (rng.random((COUT, CIN, 3, 3), dtype=np.float32) - 0.5) * 0.08
    o = kernel(x, d, w)
    print(o.shape, o.dtype)
